# revision 9
# baseline (speedup 1.0000x reference)
"""Distributed causal multi-head attention for TRN2 (8 NeuronCores).

Problem: B=2, T=2048, D=1024, H=16 heads (head_dim 64), causal MHA:
  q,k,v = x@W{q,k,v}+b, q *= dh**-0.5, o = softmax(mask(q k^T)) v, out = o@Wp + bp

Sharding: 8-way tensor parallel over heads for QKV+attention; the output
projection is re-sharded over QUERIES via one AllToAll per T-chunk.
Core r computes BOTH batches of heads {2r, 2r+1} through attention, then
ships 1/8 of its (transposed, normalized) head outputs to every peer;
after the A2A each core owns the FULL head dimension for a slice of
queries and computes all 1024 output columns for those rows.  vs an
AllGather design this moves 8x fewer collective bytes, uses the one-hop
Mesh algorithm instead of RDH, and upgrades the projection matmuls from
N=128 to N=512.  16-bit data is FP16 (rel err 6.4e-4; bf16 measures the
same speed - the PE streams 1 moving col/cycle for both - so the extra
mantissa is free margin).

Per core:
  - QKV projections on TensorE (q/k produced transposed [hd, t], v
    produced natural [t, hd] with an appended ones-column)
  - scores computed transposed [keys, q] (K=64 contraction, two heads
    packed via row tiling, both writing halves of one 2-bank PSUM
    tile); ONE fused exp per key tile on ScalarE (the pipeline pacer),
    skipping all never-read below-diagonal column blocks; causal
    handled by key-tile skipping + a post-exp 0/1 mask multiply on the
    diagonal blocks
  - AV uses exp-weights as the stationary operand -> o natural [q, hd]
    (tiny-N matmuls issue at ~36ns with LDWEIGHTS hidden; the
    v-stationary alternative measures ~2x worse) with per-partition row
    sums for free (ones column of v); normalize with per-partition
    reciprocals; AV interleaves with scores at lag 1
  - o is PE-transposed locally (the only partition-crossing device that
    is actually cheap) into the A2A bounce layout
  - all projections run in the (short) tail; emitting them earlier is
    dangerous: a premature gathered-tile load/MM blocks the whole
    engine queue behind an in-flight collective.
Host side only shards/converts inputs, concatenates outputs, and adds
the bias terms that are mathematically output-constant (bv@Wp + bp; bk
cancels in softmax; bq is applied on device).
"""

import os
import numpy as np

B, T, D, H = 2, 2048, 1024, 16
DH = 64
NCORES = 8
HPC = H // NCORES      # heads per core = 2
CD = HPC * DH          # per-core head-dim rows = 128
P = 128
NCH = 4                # T chunks (one A2A per chunk; chunk 3 per batch)
CHUNK = T // NCH       # 512
KT = T // P            # 16 key tiles
KD = D // P            # 8 contraction tiles for the projections

_CACHE = {}

# Results of the last device run (for test harnesses): BassKernelResults
LAST_RESULT = None


def _build_nc():
    import concourse.bass as bass
    import concourse.mybir as mybir
    import concourse.tile as tile
    from concourse import bacc
    from contextlib import ExitStack

    bf = mybir.dt.float16
    f32 = mybir.dt.float32
    AF = mybir.ActivationFunctionType

    nc = bacc.Bacc("TRN2", target_bir_lowering=False, debug=False,
                   num_devices=NCORES)

    xT = nc.dram_tensor("xT", [D, B, T], bf, kind="ExternalInput").ap()
    wq = nc.dram_tensor("wq", [D, CD], bf, kind="ExternalInput").ap()
    wk = nc.dram_tensor("wk", [D, CD], bf, kind="ExternalInput").ap()
    wv = nc.dram_tensor("wv", [D, CD], bf, kind="ExternalInput").ap()
    wp = nc.dram_tensor("wp", [D, D], bf, kind="ExternalInput").ap()
    bqp = nc.dram_tensor("bqp", [P, 1], f32, kind="ExternalInput").ap()
    maskf = nc.dram_tensor("maskf", [P, P], bf, kind="ExternalInput").ap()
    ident = nc.dram_tensor("ident", [P, P], bf, kind="ExternalInput").ap()
    # per-chunk rows for this core's query slice: [chunk, 128 q, D]
    out = nc.dram_tensor("out", [NCH, P, D], f32, kind="ExternalOutput").ap()

    # A2A bounce: shard j = (batch j//4, 128-q block j%4),
    # [CD rows, 128 q] contiguous 32KB.
    obounce = nc.dram_tensor("obounce", [NCH, B, NCH, CD, P], bf).ap()
    # A2A outputs: [src core (= D-row block), CD, q] per chunk
    a2ag = [nc.dram_tensor(f"a2ag{c}", [NCORES, CD, P], bf).ap()
            for c in range(NCH)]
    warm_in = nc.dram_tensor("warm_in", [P], bf).ap()
    warm_out = nc.dram_tensor("warm_out", [NCORES, P], bf,
                              addr_space="Shared").ap()

    RG = [[0, 1, 2, 3, 4, 5, 6, 7]]

    with tile.TileContext(nc, num_cores=NCORES) as tc, ExitStack() as ctx:
        const = ctx.enter_context(tc.tile_pool(name="const", bufs=1))
        work = ctx.enter_context(tc.tile_pool(name="work", bufs=3))
        expp = ctx.enter_context(tc.tile_pool(name="expp", bufs=18))
        otkp = ctx.enter_context(tc.tile_pool(name="otkp", bufs=4))
        osbp = ctx.enter_context(tc.tile_pool(name="osbp", bufs=8))
        psum = ctx.enter_context(tc.tile_pool(name="psum", bufs=2,
                                              space="PSUM"))

        # ---- persistent SBUF ----
        xT_sb = const.tile([P, KD, B, T], bf)        # 64 KB/p
        wq_sb = const.tile([P, KD, CD], bf)
        wk_sb = const.tile([P, KD, CD], bf)
        wv_sb = const.tile([P, KD, CD], bf)
        wp_sb = const.tile([P, KD, D], bf)           # full Wp: 16 KB/p
        bq_sb = const.tile([P, 1], f32)
        mask_sb = const.tile([P, P], bf)             # 0/1 lower triangle
        ident_sb = const.tile([P, P], bf)
        qT_sb = const.tile([P, B, T], bf)            # 2 heads stacked
        kT_sb = const.tile([P, B, T], bf)
        v_sb = const.tile([P, KT, B, HPC, DH + 1], bf)

        # warmup collective FIRST in program order so it triggers as soon
        # as the engines boot: it absorbs the CC-stream entry barrier +
        # first-collective staging cost while the input DMAs stream, so
        # the stream is free when chunk 0's A2A arrives.
        nc.gpsimd.collective_compute(
            "AllGather", bass.mybir.AluOpType.bypass,
            replica_groups=RG, ins=[warm_in], outs=[warm_out])
        nc.vector.memset(v_sb[:, :, :, :, DH:DH + 1], 1.0)
        # DMA order = queue order: interleave wq/x in small slices so the
        # first projection matmuls can start as soon as ~1MB has landed
        # (HBM is chip-contended at boot: all 8 cores stream inputs at
        # once); wp (only needed by the tail projection) goes last.
        xT_r = xT.rearrange("(k p) b t -> p k b t", p=P)
        wq_r = wq.rearrange("(k p) c -> p k c", p=P)
        nc.sync.dma_start(wq_sb[:, 0:2], wq_r[:, 0:2])
        nc.sync.dma_start(xT_sb[:, 0:2, 0, 0:512], xT_r[:, 0:2, 0, 0:512])
        nc.sync.dma_start(wq_sb[:, 2:4], wq_r[:, 2:4])
        nc.sync.dma_start(xT_sb[:, 2:4, 0, 0:512], xT_r[:, 2:4, 0, 0:512])
        nc.sync.dma_start(wq_sb[:, 4:8], wq_r[:, 4:8])
        nc.sync.dma_start(xT_sb[:, 4:8, 0, 0:512], xT_r[:, 4:8, 0, 0:512])
        nc.sync.dma_start(bq_sb[:], bqp)
        nc.sync.dma_start(wk_sb[:], wk.rearrange("(k p) c -> p k c", p=P))
        nc.sync.dma_start(wv_sb[:], wv.rearrange("(k p) c -> p k c", p=P))
        nc.sync.dma_start(xT_sb[:, :, 1, 0:512], xT_r[:, :, 1, 0:512])
        nc.sync.dma_start(mask_sb[:], maskf)
        nc.sync.dma_start(ident_sb[:], ident)
        for t4 in range(1, NCH):
            for b in range(B):
                nc.sync.dma_start(
                    xT_sb[:, :, b, t4 * 512:(t4 + 1) * 512],
                    xT_r[:, :, b, t4 * 512:(t4 + 1) * 512])
        nc.sync.dma_start(wp_sb[:], wp.rearrange("(k p) c -> p k c", p=P))

        def qkv_units(t4):
            """Projection work for T-chunk t4 as a list of closures, so
            it can be drip-fed into the attention k-loop (fills the PE
            while ScalarE paces the exp pipeline)."""
            units = []

            psqk_box = {}

            def q_unit(b):
                psqk = psum.tile([P, 1024], f32, tag="big", bufs=3,
                                 name=f"psqk_{t4}_{b}")
                psqk_box[b] = psqk
                for k in range(KD):
                    nc.tensor.matmul(
                        psqk[:, 0:512], wq_sb[:, k, :],
                        xT_sb[:, k, b, t4 * 512:(t4 + 1) * 512],
                        start=(k == 0), stop=(k == KD - 1))
                nc.vector.tensor_scalar_add(
                    qT_sb[:, b, t4 * 512:(t4 + 1) * 512], psqk[:, 0:512],
                    bq_sb[:, 0:1])

            def k_unit(b):
                psqk = psqk_box[b]
                for k in range(KD):
                    nc.tensor.matmul(
                        psqk[:, 512:1024], wk_sb[:, k, :],
                        xT_sb[:, k, b, t4 * 512:(t4 + 1) * 512],
                        start=(k == 0), stop=(k == KD - 1))
                nc.vector.tensor_copy(
                    kT_sb[:, b, t4 * 512:(t4 + 1) * 512], psqk[:, 512:1024])

            def v_unit(b, tt):
                psv = psum.tile([P, 256], f32, tag="big", bufs=3,
                                name=f"psv_{tt}_{b}")
                for k in range(KD):
                    nc.tensor.matmul(
                        psv[:, :CD], xT_sb[:, k, b, tt * P:(tt + 1) * P],
                        wv_sb[:, k, :], start=(k == 0),
                        stop=(k == KD - 1))
                nc.vector.tensor_copy(
                    out=v_sb[:, tt, b, :, 0:DH],
                    in_=psv[:, :CD].rearrange("p (h d) -> p h d", h=HPC))

            for b in range(B):
                units.append((t4, b, lambda b=b: q_unit(b)))
                units.append((t4, b, lambda b=b: k_unit(b)))
                for tt in range(4 * t4, 4 * t4 + 4):
                    units.append((t4, b, lambda b=b, tt=tt: v_unit(b, tt)))
            return units

        def attention_batch(c, b, filler=None):
            """Causal attention for q-chunk c, batch b (2 heads packed).

            scores for both heads go into one [128,1024] PSUM tile
            (row-packed K=64 matmuls -> halves), one fused exp per key
            tile, AV interleaved with lag 1.  AV accumulates all four
            q-subtiles of each head in one PSUM bank (4 interleaved
            accumulation groups as column ranges)."""
            nkt = 4 * (c + 1)
            exp_tiles = {}
            pso = {}
            for hh in range(2):
                pso[hh] = psum.tile([P, 4, DH + 1], f32, tag="o",
                                    name=f"pso_{c}_{b}_{hh}")

            def do_scores(k):
                j = k - 4 * c
                ql = max(j, 0) * P   # causal: only queries >= key tile
                ps_s = psum.tile([P, 1024], f32, tag="big", bufs=3,
                                 name=f"ps_{c}_{b}_{k}")
                for hh in range(2):
                    lo, hi = hh * DH, (hh + 1) * DH
                    nc.tensor.matmul(
                        ps_s[:, hh * 512 + ql:(hh + 1) * 512],
                        kT_sb[lo:hi, b, k * P:(k + 1) * P],
                        qT_sb[lo:hi, b, c * 512 + ql:(c + 1) * 512],
                        start=True, stop=True)
                e = expp.tile([P, 1024], bf, tag="expT",
                              name=f"expT_{c}_{b}_{k}")
                # diagonal tiles: below-diagonal column blocks are never
                # read by AV, but each ACT op has a ~352cyc fixed pipe
                # cost - only skip when the skipped width exceeds it.
                # (exp of stale PSUM cols lands in never-read SBUF.)
                if j == 3:
                    for hh in range(2):
                        lo = hh * 512 + j * P
                        hi = (hh + 1) * 512
                        nc.scalar.activation(e[:, lo:hi], ps_s[:, lo:hi],
                                             AF.Exp)
                elif j == 2:
                    nc.scalar.activation(e[:, 256:1024], ps_s[:, 256:1024],
                                         AF.Exp)
                else:
                    nc.scalar.activation(e[:], ps_s[:], AF.Exp)
                if j >= 0:
                    blks = e[:].rearrange("p (hh q) -> p hh q", hh=2)[
                        :, :, j * P:(j + 1) * P]
                    nc.vector.tensor_mul(
                        blks, blks,
                        mask_sb[:, None, :].to_broadcast([P, 2, P]))
                exp_tiles[k] = e

            def do_av(k):
                # pso[hh] holds 4 interleaved accumulation groups in one
                # PSUM bank; only the first write of the bank (k==0,s==0)
                # may set start (bank-wide has_written clear).  For diag
                # key tiles (k>0) the mask-dependent s==j block goes last
                # so the other AV matmuls never queue behind the DVE mask.
                j = k - 4 * c
                order = list(range(4))
                if k > 0 and 0 <= j < 4:
                    order = [s for s in order if s != j] + [j]
                for hh in range(2):
                    for s in order:
                        if k <= 4 * c + s:
                            nc.tensor.matmul(
                                pso[hh][:, s, :],
                                exp_tiles[k][:, hh * 512 + s * P:
                                             hh * 512 + (s + 1) * P],
                                v_sb[:, k, b, hh, :],
                                start=(k == 0 and s == 0),
                                stop=(k == 4 * c + s),
                                skip_group_check=True)

            for k in range(nkt + 1):
                if k < nkt:
                    do_scores(k)
                if k > 0:
                    do_av(k - 1)
                if filler is not None and k >= 2 and (c == 0 or
                                                     k < nkt - 1):
                    # no fills near the end of the loop (except chunk 0,
                    # which feeds no collective): the A2A-critical finish
                    # work must not queue behind drip units
                    filler()
            return pso

        def proj_units(c):
            """Output projection for T-chunk c from the gathered A2A
            slice: this core's 128 query rows x all 1024 output columns.
            Emitted only in the tail, when the A2A is (nearly) done."""
            oT_sb = otkp.tile([P, KD, P], bf, tag="oTk", name=f"oTk_{c}")

            def load():
                g = a2ag[c].rearrange("k r q -> r k q")
                nc.sync.dma_start(oT_sb[:, 0:4], g[:, 0:4])
                nc.sync.dma_start(oT_sb[:, 4:8], g[:, 4:8])

            psp_box = {}

            def mms(n):
                if n == 0:
                    psp_box[0] = psum.tile([P, 2, 512], f32, tag="big",
                                           bufs=3, name=f"psp_{c}")
                psp = psp_box[0]
                for k in range(KD):
                    nc.tensor.matmul(
                        psp[:, n, :], oT_sb[:, k, :],
                        wp_sb[:, k, n * 512:(n + 1) * 512],
                        start=(k == 0), stop=(k == KD - 1))

            def fin():
                psp = psp_box[0]
                outsb = work.tile([P, 2, 512], f32, tag="outsb",
                                  name=f"outsb_{c}")
                for n in range(2):
                    nc.vector.tensor_copy(outsb[:, n, :], psp[:, n, :])
                    nc.sync.dma_start(
                        out[c][:, n * 512:(n + 1) * 512], outsb[:, n, :])

            return [load, lambda: mms(0), lambda: mms(1), fin]

        def finish_batch(c, b, pso):
            """normalize -> PE transpose -> A2A bounce buffer."""
            osb = osbp.tile([P, 4, CD], bf, tag="osb",
                            name=f"osb_{c}_{b}")
            for hh in range(2):
                rec = work.tile([P, 4], f32, tag="rec",
                                name=f"rec_{c}_{b}_{hh}")
                nc.vector.reciprocal(rec[:], pso[hh][:, :, DH:DH + 1])
                for s in range(4):
                    nc.vector.tensor_scalar_mul(
                        osb[:, s, hh * DH:(hh + 1) * DH],
                        pso[hh][:, s, 0:DH], rec[:, s:s + 1])
            # local PE transpose: the bounce carries oT so the projection
            # needs no DMA transposes
            obT = work.tile([P, 4, P], bf, tag="obT", name=f"obT_{c}_{b}")
            for s in range(4):
                trp = psum.tile([P, P], bf, tag="o",
                                name=f"trp_{c}_{b}_{s}")
                nc.tensor.transpose(trp[:], osb[:, s, :], ident_sb[:])
                nc.vector.tensor_copy(obT[:, s, :], trp[:])
            # [row, (qblk q)] -> bounce [qblk, row, q]
            nc.sync.dma_start(
                obounce[c, b].rearrange("s r q -> r s q"), obT[:])

        # pipeline: attention(c) paces ScalarE; one A2A per chunk flies
        # while later chunks compute; qkv(c+1) is drip-fed INTO the
        # attention k-loop so the PE fills ScalarE-paced slack.
        pending = []   # (chunk, batch, closure) drip units

        def filler():
            if pending:
                pending.pop(0)[2]()
            if len(pending) > 8:
                pending.pop(0)[2]()

        def drain_for(c, b):
            while any(t == c and bb == b for t, bb, _ in pending):
                pending.pop(0)[2]()

        # only batch 0's projections block the first scores
        units0 = qkv_units(0)
        for _, _, u in units0[:6]:
            u()
        pending.extend(units0[6:])
        for c in range(NCH):
            if c + 1 < NCH:
                pending.extend(qkv_units(c + 1))
            if c == 3:
                # A2As 0 and 1 completed ~30us ago: their projections can
                # safely drip into chunk 3's ScalarE-paced slack
                for cc in (0, 1):
                    pending.extend((-1, -1, u) for u in proj_units(cc))
            for b in range(B):
                drain_for(c, b)
                pso = attention_batch(c, b, filler=filler)
                finish_batch(c, b, pso)
            nc.gpsimd.collective_compute(
                "AllToAll", bass.mybir.AluOpType.bypass,
                replica_groups=RG, ins=[obounce[c]], outs=[a2ag[c]])
        while pending:
            pending.pop(0)[2]()
        # tail: chunk 2's projection runs immediately; chunk 3's waits on
        # the last A2A.  The wait is long (slowest-peer skew), so bridging
        # matmuls can't keep the clock warm through it - run the final
        # projection cold rather than burn PE power on a throwaway bridge.
        for u in proj_units(2):
            u()
        for u in proj_units(3):
            u()

    nc.finalize()
    return nc


def _get_nc():
    if "nc" not in _CACHE:
        _CACHE["nc"] = _build_nc()
    return _CACHE["nc"]


def kernel(x, Wq, bq, Wk, bk, Wv, bv, Wp, bp):
    global LAST_RESULT
    from concourse.bass_utils import run_bass_kernel_spmd

    bf16 = np.float16
    x = np.asarray(x, dtype=np.float32)
    Wq = np.asarray(Wq, dtype=np.float32)
    Wk = np.asarray(Wk, dtype=np.float32)
    Wv = np.asarray(Wv, dtype=np.float32)
    Wp = np.asarray(Wp, dtype=np.float32)
    bq = np.asarray(bq, dtype=np.float32)
    bv = np.asarray(bv, dtype=np.float32)
    bp = np.asarray(bp, dtype=np.float32)

    s = DH ** -0.5
    maskf = np.where(
        np.arange(P)[:, None] <= np.arange(P)[None, :], 1.0, 0.0
    ).astype(bf16)
    ident = np.eye(P, dtype=bf16)
    xTg = np.ascontiguousarray(np.stack([x[0].T, x[1].T], axis=1)
                               ).astype(bf16)
    wp16 = np.ascontiguousarray(Wp).astype(bf16)

    in_maps = []
    for r in range(NCORES):
        cols = slice(r * CD, (r + 1) * CD)
        in_maps.append({
            "xT": xTg,
            "wq": (Wq[:, cols] * s).astype(bf16),
            "wk": np.ascontiguousarray(Wk[:, cols]).astype(bf16),
            "wv": np.ascontiguousarray(Wv[:, cols]).astype(bf16),
            "wp": wp16,
            "bqp": np.ascontiguousarray((bq[cols] * s).reshape(P, 1)),
            "maskf": maskf,
            "ident": ident,
        })

    nc = _get_nc()
    extra = {}
    if os.environ.get("KERNEL_TRACE_ALL"):
        extra["trace_cores"] = list(range(NCORES))
    res = run_bass_kernel_spmd(
        nc, in_maps, core_ids=list(range(NCORES)),
        trace=bool(int(os.environ.get("KERNEL_TRACE", "0"))), **extra)
    LAST_RESULT = res

    out = np.empty((B, T, D), dtype=np.float32)
    for r in range(NCORES):
        o = res.results[r]["out"]          # [NCH, 128, 1024]
        br, blk = r // NCH, r % NCH
        for c in range(NCH):
            out[br, c * CHUNK + blk * P:c * CHUNK + (blk + 1) * P, :] = o[c]
    # bias terms that are constant w.r.t. the data path:
    #   v-bias passes through softmax rows (sum=1) -> + bv@Wp; plus bp.
    #   (bk shifts every logit in a row equally -> cancels in softmax.)
    out += (bv @ Wp + bp)[None, None, :]
    return out



# revision 14
# speedup vs baseline: 1.4399x; 1.4399x over previous
"""Distributed causal multi-head attention for TRN2 (8 NeuronCores).

Problem: B=2, T=2048, D=1024, H=16 heads (head_dim 64), causal MHA:
  q,k,v = x@W{q,k,v}+b, q *= dh**-0.5, o = softmax(mask(q k^T)) v, out = o@Wp + bp

Sharding: 8-way tensor parallel over heads for QKV+attention; the output
projection is re-sharded over QUERIES via one AllToAll per T-chunk.
Core r computes BOTH batches of heads {2r, 2r+1} through attention, then
ships 1/8 of its (transposed, normalized) head outputs to every peer;
after the A2A each core owns the FULL head dimension for a slice of
queries and computes all 1024 output columns for those rows.  vs an
AllGather design this moves 8x fewer collective bytes, uses the one-hop
Mesh algorithm instead of RDH, and upgrades the projection matmuls from
N=128 to N=512.  16-bit data is FP16 (rel err 6.4e-4; bf16 measures the
same speed - the PE streams 1 moving col/cycle for both - so the extra
mantissa is free margin).

Per core:
  - QKV projections on TensorE (q/k produced transposed [hd, t], v
    produced natural [t, hd] with an appended ones-column)
  - scores computed transposed [keys, q] (K=64 contraction, two heads
    packed via row tiling, both writing halves of one 2-bank PSUM
    tile); ONE fused exp per key tile on ScalarE (the pipeline pacer),
    skipping all never-read below-diagonal column blocks; causal
    handled by key-tile skipping + a post-exp 0/1 mask multiply on the
    diagonal blocks
  - AV uses exp-weights as the stationary operand -> o natural [q, hd]
    (tiny-N matmuls issue at ~36ns with LDWEIGHTS hidden; the
    v-stationary alternative measures ~2x worse) with per-partition row
    sums for free (ones column of v); normalize with per-partition
    reciprocals; AV interleaves with scores at lag 1
  - o is PE-transposed locally (the only partition-crossing device that
    is actually cheap) into the A2A bounce layout
  - all projections run in the (short) tail; emitting them earlier is
    dangerous: a premature gathered-tile load/MM blocks the whole
    engine queue behind an in-flight collective.
Host side only shards/converts inputs, concatenates outputs, and adds
the bias terms that are mathematically output-constant (bv@Wp + bp; bk
cancels in softmax; bq is applied on device).
"""

import os
import numpy as np

B, T, D, H = 2, 2048, 1024, 16
DH = 64
NCORES = 8
HPC = H // NCORES      # heads per core = 2
CD = HPC * DH          # per-core head-dim rows = 128
P = 128
NCH = 4                # T chunks (one A2A per chunk; chunk 3 per batch)
CHUNK = T // NCH       # 512
KT = T // P            # 16 key tiles
KD = D // P            # 8 contraction tiles for the projections

_CACHE = {}

# Results of the last device run (for test harnesses): BassKernelResults
LAST_RESULT = None


def _build_nc():
    import concourse.bass as bass
    import concourse.mybir as mybir
    import concourse.tile as tile
    from concourse import bacc
    from contextlib import ExitStack

    bf = mybir.dt.float16
    f32 = mybir.dt.float32
    AF = mybir.ActivationFunctionType

    nc = bacc.Bacc("TRN2", target_bir_lowering=False, debug=False,
                   num_devices=NCORES)

    xT = nc.dram_tensor("xT", [D, B, T], bf, kind="ExternalInput").ap()
    wq = nc.dram_tensor("wq", [D, CD], bf, kind="ExternalInput").ap()
    wk = nc.dram_tensor("wk", [D, CD], bf, kind="ExternalInput").ap()
    wv = nc.dram_tensor("wv", [D, CD], bf, kind="ExternalInput").ap()
    wp = nc.dram_tensor("wp", [D, D], bf, kind="ExternalInput").ap()
    bqp = nc.dram_tensor("bqp", [P, 1], f32, kind="ExternalInput").ap()
    maskf = nc.dram_tensor("maskf", [P, P], bf, kind="ExternalInput").ap()
    ident = nc.dram_tensor("ident", [P, P], bf, kind="ExternalInput").ap()
    # per-chunk rows for this core's query slice: [chunk, 128 q, D]
    out = nc.dram_tensor("out", [NCH, P, D], f32, kind="ExternalOutput").ap()

    # A2A bounce: shard j = (batch j//4, 128-q block j%4),
    # [CD rows, 128 q] contiguous 32KB.
    obounce = nc.dram_tensor("obounce", [NCH, B, NCH, CD, P], bf).ap()
    # A2A outputs: [src core (= D-row block), CD, q] per chunk
    a2ag = [nc.dram_tensor(f"a2ag{c}", [NCORES, CD, P], bf).ap()
            for c in range(NCH)]
    warm_in = nc.dram_tensor("warm_in", [P], bf).ap()
    warm_out = nc.dram_tensor("warm_out", [NCORES, P], bf,
                              addr_space="Shared").ap()

    RG = [[0, 1, 2, 3, 4, 5, 6, 7]]

    with tile.TileContext(nc, num_cores=NCORES) as tc, ExitStack() as ctx:
        const = ctx.enter_context(tc.tile_pool(name="const", bufs=1))
        work = ctx.enter_context(tc.tile_pool(name="work", bufs=3))
        expp = ctx.enter_context(tc.tile_pool(name="expp", bufs=18))
        otkp = ctx.enter_context(tc.tile_pool(name="otkp", bufs=4))
        osbp = ctx.enter_context(tc.tile_pool(name="osbp", bufs=8))
        psum = ctx.enter_context(tc.tile_pool(name="psum", bufs=2,
                                              space="PSUM"))

        # ---- persistent SBUF ----
        xT_sb = const.tile([P, KD, B, T], bf)        # 64 KB/p
        wq_sb = const.tile([P, KD, CD], bf)
        wk_sb = const.tile([P, KD, CD], bf)
        wv_sb = const.tile([P, KD, CD], bf)
        wp_sb = const.tile([P, KD, D], bf)           # full Wp: 16 KB/p
        bq_sb = const.tile([P, 1], f32)
        mask_sb = const.tile([P, P], bf)             # 0/1 lower triangle
        ident_sb = const.tile([P, P], bf)
        qT_sb = const.tile([P, B, T], bf)            # 2 heads stacked
        kT_sb = const.tile([P, B, T], bf)
        v_sb = const.tile([P, KT, B, HPC, DH + 1], bf)

        # warmup collective FIRST in program order so it triggers as soon
        # as the engines boot: it absorbs the CC-stream entry barrier +
        # first-collective staging cost while the input DMAs stream, so
        # the stream is free when chunk 0's A2A arrives.
        nc.gpsimd.collective_compute(
            "AllGather", bass.mybir.AluOpType.bypass,
            replica_groups=RG, ins=[warm_in], outs=[warm_out])
        nc.vector.memset(v_sb[:, :, :, :, DH:DH + 1], 1.0)
        # DMA order = queue order: interleave wq/x in small slices so the
        # first projection matmuls can start as soon as ~1MB has landed
        # (HBM is chip-contended at boot: all 8 cores stream inputs at
        # once); wp (only needed by the tail projection) goes last.
        xT_r = xT.rearrange("(k p) b t -> p k b t", p=P)
        wq_r = wq.rearrange("(k p) c -> p k c", p=P)
        nc.sync.dma_start(wq_sb[:, 0:1], wq_r[:, 0:1])
        nc.sync.dma_start(xT_sb[:, 0:1, 0, 0:512], xT_r[:, 0:1, 0, 0:512])
        nc.sync.dma_start(wq_sb[:, 1:2], wq_r[:, 1:2])
        nc.sync.dma_start(xT_sb[:, 1:2, 0, 0:512], xT_r[:, 1:2, 0, 0:512])
        nc.sync.dma_start(wq_sb[:, 2:4], wq_r[:, 2:4])
        nc.sync.dma_start(xT_sb[:, 2:4, 0, 0:512], xT_r[:, 2:4, 0, 0:512])
        nc.sync.dma_start(wq_sb[:, 4:8], wq_r[:, 4:8])
        nc.sync.dma_start(xT_sb[:, 4:8, 0, 0:512], xT_r[:, 4:8, 0, 0:512])
        nc.sync.dma_start(bq_sb[:], bqp)
        nc.sync.dma_start(wk_sb[:], wk.rearrange("(k p) c -> p k c", p=P))
        nc.sync.dma_start(wv_sb[:], wv.rearrange("(k p) c -> p k c", p=P))
        nc.sync.dma_start(xT_sb[:, :, 1, 0:512], xT_r[:, :, 1, 0:512])
        nc.sync.dma_start(mask_sb[:], maskf)
        nc.sync.dma_start(ident_sb[:], ident)
        for t4 in range(1, NCH):
            for b in range(B):
                nc.sync.dma_start(
                    xT_sb[:, :, b, t4 * 512:(t4 + 1) * 512],
                    xT_r[:, :, b, t4 * 512:(t4 + 1) * 512])
        nc.sync.dma_start(wp_sb[:], wp.rearrange("(k p) c -> p k c", p=P))

        def qkv_units(t4):
            """Projection work for T-chunk t4 as a list of closures, so
            it can be drip-fed into the attention k-loop (fills the PE
            while ScalarE paces the exp pipeline)."""
            units = []

            psqk_box = {}

            def q_unit(b):
                psqk = psum.tile([P, 1024], f32, tag="big", bufs=3,
                                 name=f"psqk_{t4}_{b}")
                psqk_box[b] = psqk
                for k in range(KD):
                    nc.tensor.matmul(
                        psqk[:, 0:512], wq_sb[:, k, :],
                        xT_sb[:, k, b, t4 * 512:(t4 + 1) * 512],
                        start=(k == 0), stop=(k == KD - 1))
                nc.vector.tensor_scalar_add(
                    qT_sb[:, b, t4 * 512:(t4 + 1) * 512], psqk[:, 0:512],
                    bq_sb[:, 0:1])

            def k_unit(b):
                psqk = psqk_box[b]
                for k in range(KD):
                    nc.tensor.matmul(
                        psqk[:, 512:1024], wk_sb[:, k, :],
                        xT_sb[:, k, b, t4 * 512:(t4 + 1) * 512],
                        start=(k == 0), stop=(k == KD - 1))
                nc.vector.tensor_copy(
                    kT_sb[:, b, t4 * 512:(t4 + 1) * 512], psqk[:, 512:1024])

            def v_unit(b, tt):
                psv = psum.tile([P, 256], f32, tag="big", bufs=3,
                                name=f"psv_{tt}_{b}")
                for k in range(KD):
                    nc.tensor.matmul(
                        psv[:, :CD], xT_sb[:, k, b, tt * P:(tt + 1) * P],
                        wv_sb[:, k, :], start=(k == 0),
                        stop=(k == KD - 1))
                nc.vector.tensor_copy(
                    out=v_sb[:, tt, b, :, 0:DH],
                    in_=psv[:, :CD].rearrange("p (h d) -> p h d", h=HPC))

            for b in range(B):
                units.append((t4, b, lambda b=b: q_unit(b)))
                units.append((t4, b, lambda b=b: k_unit(b)))
                for tt in range(4 * t4, 4 * t4 + 4):
                    units.append((t4, b, lambda b=b, tt=tt: v_unit(b, tt)))
            return units

        def attention_batch(c, b, filler=None, early=None):
            """Causal attention for q-chunk c, batch b (2 heads packed).

            scores for both heads go into one [128,1024] PSUM tile
            (row-packed K=64 matmuls -> halves), one fused exp per key
            tile, AV interleaved with lag 1.  AV accumulates all four
            q-subtiles of each head in one PSUM bank (4 interleaved
            accumulation groups as column ranges).

            `early` (the previous batch's finish + A2A post) is emitted
            after the first two score tiles so its PE transposes overlap
            the DVE normalize instead of stalling the next batch's
            scores in the Tensor queue.  pso is allocated lazily AFTER
            `early` so the tag-"o" PSUM rotation reuses only buffers
            whose readers are already emitted."""
            nkt = 4 * (c + 1)
            exp_tiles = {}
            pso = {}

            def ensure_pso():
                for hh in range(2):
                    pso[hh] = psum.tile([P, 4, DH + 1], f32, tag="o",
                                        name=f"pso_{c}_{b}_{hh}")

            def do_scores(k):
                j = k - 4 * c
                ql = max(j, 0) * P   # causal: only queries >= key tile
                ps_s = psum.tile([P, 1024], f32, tag="big", bufs=3,
                                 name=f"ps_{c}_{b}_{k}")
                for hh in range(2):
                    lo, hi = hh * DH, (hh + 1) * DH
                    nc.tensor.matmul(
                        ps_s[:, hh * 512 + ql:(hh + 1) * 512],
                        kT_sb[lo:hi, b, k * P:(k + 1) * P],
                        qT_sb[lo:hi, b, c * 512 + ql:(c + 1) * 512],
                        start=True, stop=True)
                e = expp.tile([P, 1024], bf, tag="expT",
                              name=f"expT_{c}_{b}_{k}")
                # diagonal tiles: below-diagonal column blocks are never
                # read by AV, but each ACT op has a ~352cyc fixed pipe
                # cost - only skip when the skipped width exceeds it.
                # (exp of stale PSUM cols lands in never-read SBUF.)
                if j == 3:
                    for hh in range(2):
                        lo = hh * 512 + j * P
                        hi = (hh + 1) * 512
                        nc.scalar.activation(e[:, lo:hi], ps_s[:, lo:hi],
                                             AF.Exp)
                elif j == 2:
                    nc.scalar.activation(e[:, 256:1024], ps_s[:, 256:1024],
                                         AF.Exp)
                else:
                    nc.scalar.activation(e[:], ps_s[:], AF.Exp)
                if j >= 0:
                    blks = e[:].rearrange("p (hh q) -> p hh q", hh=2)[
                        :, :, j * P:(j + 1) * P]
                    nc.vector.tensor_mul(
                        blks, blks,
                        mask_sb[:, None, :].to_broadcast([P, 2, P]))
                exp_tiles[k] = e

            def do_av(k):
                # pso[hh] holds 4 interleaved accumulation groups in one
                # PSUM bank; only the first write of the bank (k==0,s==0)
                # may set start (bank-wide has_written clear).  For diag
                # key tiles (k>0) the mask-dependent s==j block goes last
                # so the other AV matmuls never queue behind the DVE mask.
                j = k - 4 * c
                order = list(range(4))
                if k > 0 and 0 <= j < 4:
                    order = [s for s in order if s != j] + [j]
                for hh in range(2):
                    for s in order:
                        if k <= 4 * c + s:
                            nc.tensor.matmul(
                                pso[hh][:, s, :],
                                exp_tiles[k][:, hh * 512 + s * P:
                                             hh * 512 + (s + 1) * P],
                                v_sb[:, k, b, hh, :],
                                start=(k == 0 and s == 0),
                                stop=(k == 4 * c + s),
                                skip_group_check=True)

            do_scores(0)
            do_scores(1)
            if early is not None:
                early()
            ensure_pso()
            do_av(0)
            for k in range(2, nkt + 1):
                if k < nkt:
                    do_scores(k)
                do_av(k - 1)
                if filler is not None and (c == 0 or k < nkt - 1):
                    # no fills near the end of the loop (except chunk 0,
                    # which feeds no collective): the A2A-critical finish
                    # work must not queue behind drip units
                    filler()
            return pso

        def proj_units(c):
            """Output projection for T-chunk c from the gathered A2A
            slice: this core's 128 query rows x all 1024 output columns.
            Emitted only in the tail, when the A2A is (nearly) done."""
            oT_sb = otkp.tile([P, KD, P], bf, tag="oTk", name=f"oTk_{c}")

            def load():
                g = a2ag[c].rearrange("k r q -> r k q")
                nc.sync.dma_start(oT_sb[:, 0:4], g[:, 0:4])
                nc.sync.dma_start(oT_sb[:, 4:8], g[:, 4:8])

            psp_box = {}

            def mms(n):
                if n == 0:
                    psp_box[0] = psum.tile([P, 2, 512], f32, tag="big",
                                           bufs=3, name=f"psp_{c}")
                psp = psp_box[0]
                for k in range(KD):
                    nc.tensor.matmul(
                        psp[:, n, :], oT_sb[:, k, :],
                        wp_sb[:, k, n * 512:(n + 1) * 512],
                        start=(k == 0), stop=(k == KD - 1))

            def fin():
                psp = psp_box[0]
                outsb = work.tile([P, 2, 512], f32, tag="outsb",
                                  name=f"outsb_{c}")
                for n in range(2):
                    nc.vector.tensor_copy(outsb[:, n, :], psp[:, n, :])
                    nc.sync.dma_start(
                        out[c][:, n * 512:(n + 1) * 512], outsb[:, n, :])

            return [load, lambda: mms(0), lambda: mms(1), fin]

        def finish_batch(c, b, pso):
            """normalize -> PE transpose -> A2A bounce buffer."""
            osb = osbp.tile([P, 4, CD], bf, tag="osb",
                            name=f"osb_{c}_{b}")
            for hh in range(2):
                rec = work.tile([P, 4], f32, tag="rec",
                                name=f"rec_{c}_{b}_{hh}")
                nc.vector.reciprocal(rec[:], pso[hh][:, :, DH:DH + 1])
                for s in range(4):
                    nc.vector.tensor_scalar_mul(
                        osb[:, s, hh * DH:(hh + 1) * DH],
                        pso[hh][:, s, 0:DH], rec[:, s:s + 1])
            # local PE transpose: the bounce carries oT so the projection
            # needs no DMA transposes.  All 4 transposes land in one PSUM
            # tile (1KB, fits the tag-"o" slot) -> one contiguous copy.
            obT = work.tile([P, 4, P], bf, tag="obT", name=f"obT_{c}_{b}")
            trp4 = psum.tile([P, 4, P], bf, tag="o", name=f"trp_{c}_{b}")
            for s in range(4):
                nc.tensor.transpose(trp4[:, s, :], osb[:, s, :],
                                    ident_sb[:])
            nc.vector.tensor_copy(obT[:], trp4[:])
            # [row, (qblk q)] -> bounce [qblk, row, q]
            nc.sync.dma_start(
                obounce[c, b].rearrange("s r q -> r s q"), obT[:])

        # pipeline: attention(c) paces ScalarE; one A2A per chunk flies
        # while later chunks compute; qkv(c+1) is drip-fed INTO the
        # attention k-loop so the PE fills ScalarE-paced slack.
        pending = []   # (chunk, batch, closure) drip units

        def filler():
            if pending:
                pending.pop(0)[2]()
            if len(pending) > 8:
                pending.pop(0)[2]()

        def drain_for(c, b):
            while any(t == c and bb == b for t, bb, _ in pending):
                pending.pop(0)[2]()

        def post_a2a(c):
            nc.gpsimd.collective_compute(
                "AllToAll", bass.mybir.AluOpType.bypass,
                replica_groups=RG, ins=[obounce[c]], outs=[a2ag[c]])

        # only batch 0's projections block the first scores
        units0 = qkv_units(0)
        for _, _, u in units0[:6]:
            u()
        pending.extend(units0[6:])
        prevbox = []

        def make_early(snapshot):
            def early():
                for pc, pb, ppso in snapshot:
                    finish_batch(pc, pb, ppso)
                    if pb == B - 1:
                        post_a2a(pc)
            return early

        for c in range(NCH):
            if c + 1 < NCH:
                pending.extend(qkv_units(c + 1))
            if c == 3:
                # A2As 0 and 1 completed ~30us ago: their projections can
                # safely drip into chunk 3's ScalarE-paced slack
                for cc in (0, 1):
                    pending.extend((-1, -1, u) for u in proj_units(cc))
            for b in range(B):
                drain_for(c, b)
                early = make_early(list(prevbox)) if prevbox else None
                prevbox.clear()
                pso = attention_batch(c, b, filler=filler, early=early)
                prevbox.append((c, b, pso))
        # the last batch's finish + A2A are emitted immediately: nothing
        # may delay the final collective
        for pc, pb, ppso in prevbox:
            finish_batch(pc, pb, ppso)
            post_a2a(pc)
        while pending:
            pending.pop(0)[2]()
        # tail: chunk 2's projection runs immediately; chunk 3's waits on
        # the last A2A.  The wait is long (slowest-peer skew), so bridging
        # matmuls can't keep the clock warm through it - run the final
        # projection cold rather than burn PE power on a throwaway bridge.
        for u in proj_units(2):
            u()
        for u in proj_units(3):
            u()

    nc.finalize()
    return nc


def _get_nc():
    if "nc" not in _CACHE:
        _CACHE["nc"] = _build_nc()
    return _CACHE["nc"]


def kernel(x, Wq, bq, Wk, bk, Wv, bv, Wp, bp):
    global LAST_RESULT
    from concourse.bass_utils import run_bass_kernel_spmd

    bf16 = np.float16
    x = np.asarray(x, dtype=np.float32)
    Wq = np.asarray(Wq, dtype=np.float32)
    Wk = np.asarray(Wk, dtype=np.float32)
    Wv = np.asarray(Wv, dtype=np.float32)
    Wp = np.asarray(Wp, dtype=np.float32)
    bq = np.asarray(bq, dtype=np.float32)
    bv = np.asarray(bv, dtype=np.float32)
    bp = np.asarray(bp, dtype=np.float32)

    s = DH ** -0.5
    maskf = np.where(
        np.arange(P)[:, None] <= np.arange(P)[None, :], 1.0, 0.0
    ).astype(bf16)
    ident = np.eye(P, dtype=bf16)
    xTg = np.ascontiguousarray(np.stack([x[0].T, x[1].T], axis=1)
                               ).astype(bf16)
    wp16 = np.ascontiguousarray(Wp).astype(bf16)

    in_maps = []
    for r in range(NCORES):
        cols = slice(r * CD, (r + 1) * CD)
        in_maps.append({
            "xT": xTg,
            "wq": (Wq[:, cols] * s).astype(bf16),
            "wk": np.ascontiguousarray(Wk[:, cols]).astype(bf16),
            "wv": np.ascontiguousarray(Wv[:, cols]).astype(bf16),
            "wp": wp16,
            "bqp": np.ascontiguousarray((bq[cols] * s).reshape(P, 1)),
            "maskf": maskf,
            "ident": ident,
        })

    nc = _get_nc()
    extra = {}
    if os.environ.get("KERNEL_TRACE_ALL"):
        extra["trace_cores"] = list(range(NCORES))
    res = run_bass_kernel_spmd(
        nc, in_maps, core_ids=list(range(NCORES)),
        trace=bool(int(os.environ.get("KERNEL_TRACE", "0"))), **extra)
    LAST_RESULT = res

    out = np.empty((B, T, D), dtype=np.float32)
    for r in range(NCORES):
        o = res.results[r]["out"]          # [NCH, 128, 1024]
        br, blk = r // NCH, r % NCH
        for c in range(NCH):
            out[br, c * CHUNK + blk * P:c * CHUNK + (blk + 1) * P, :] = o[c]
    # bias terms that are constant w.r.t. the data path:
    #   v-bias passes through softmax rows (sum=1) -> + bv@Wp; plus bp.
    #   (bk shifts every logit in a row equally -> cancels in softmax.)
    out += (bv @ Wp + bp)[None, None, :]
    return out



# revision 21
# speedup vs baseline: 1.5203x; 1.0558x over previous
"""Distributed causal multi-head attention for TRN2 (8 NeuronCores).

Problem: B=2, T=2048, D=1024, H=16 heads (head_dim 64), causal MHA:
  q,k,v = x@W{q,k,v}+b, q *= dh**-0.5, o = softmax(mask(q k^T)) v, out = o@Wp + bp

Sharding: 8-way tensor parallel over heads for QKV+attention; the output
projection is re-sharded over QUERIES via one AllToAll per T-chunk.
Core r computes BOTH batches of heads {2r, 2r+1} through attention, then
ships 1/8 of its (transposed, normalized) head outputs to every peer;
after the A2A each core owns the FULL head dimension for a slice of
queries and computes all 1024 output columns for those rows.  vs an
AllGather design this moves 8x fewer collective bytes, uses the one-hop
Mesh algorithm instead of RDH, and upgrades the projection matmuls from
N=128 to N=512.  16-bit data is FP16 (rel err 6.4e-4; bf16 measures the
same speed - the PE streams 1 moving col/cycle for both - so the extra
mantissa is free margin).

Per core:
  - QKV projections on TensorE (q/k produced transposed [hd, t], v
    produced natural [t, hd] with an appended ones-column)
  - scores computed transposed [keys, q] (K=64 contraction, two heads
    packed via row tiling, both writing halves of one 2-bank PSUM
    tile); ONE fused exp per key tile on ScalarE (the pipeline pacer),
    skipping all never-read below-diagonal column blocks; causal
    handled by key-tile skipping + a post-exp 0/1 mask multiply on the
    diagonal blocks
  - AV uses exp-weights as the stationary operand -> o natural [q, hd]
    (tiny-N matmuls issue at ~36ns with LDWEIGHTS hidden; the
    v-stationary alternative measures ~2x worse) with per-partition row
    sums for free (ones column of v); normalize with per-partition
    reciprocals; AV interleaves with scores at lag 1
  - o is PE-transposed locally (the only partition-crossing device that
    is actually cheap) into the A2A bounce layout
  - all projections run in the (short) tail; emitting them earlier is
    dangerous: a premature gathered-tile load/MM blocks the whole
    engine queue behind an in-flight collective.
Host side only shards/converts inputs, concatenates outputs, and adds
the bias terms that are mathematically output-constant (bv@Wp + bp; bk
cancels in softmax; bq is applied on device).
"""

import os
import numpy as np

B, T, D, H = 2, 2048, 1024, 16
DH = 64
NCORES = 8
HPC = H // NCORES      # heads per core = 2
CD = HPC * DH          # per-core head-dim rows = 128
P = 128
NCH = 4                # T chunks (one A2A per chunk; chunk 3 per batch)
CHUNK = T // NCH       # 512
KT = T // P            # 16 key tiles
KD = D // P            # 8 contraction tiles for the projections

_CACHE = {}

# Results of the last device run (for test harnesses): BassKernelResults
LAST_RESULT = None


def _build_nc():
    import concourse.bass as bass
    import concourse.mybir as mybir
    import concourse.tile as tile
    from concourse import bacc
    from contextlib import ExitStack

    bf = mybir.dt.float16
    f32 = mybir.dt.float32
    AF = mybir.ActivationFunctionType

    nc = bacc.Bacc("TRN2", target_bir_lowering=False, debug=False,
                   num_devices=NCORES)

    xT = nc.dram_tensor("xT", [D, B, T], bf, kind="ExternalInput").ap()
    wq = nc.dram_tensor("wq", [D, CD], bf, kind="ExternalInput").ap()
    wk = nc.dram_tensor("wk", [D, CD], bf, kind="ExternalInput").ap()
    wv = nc.dram_tensor("wv", [D, CD], bf, kind="ExternalInput").ap()
    wp = nc.dram_tensor("wp", [D, D], bf, kind="ExternalInput").ap()
    bqp = nc.dram_tensor("bqp", [P, 1], f32, kind="ExternalInput").ap()
    maskf = nc.dram_tensor("maskf", [P, P], bf, kind="ExternalInput").ap()
    ident = nc.dram_tensor("ident", [P, P], bf, kind="ExternalInput").ap()
    # per-chunk rows for this core's query slice: [chunk, 128 q, D].
    # bf16 store: the o@Wp values already carry bf16-operand noise, the
    # extra store rounding is ~0.2% rms vs the 2e-2 gate; halves the
    # tail's output DMA and speeds the PSUM->SBUF copies.
    out = nc.dram_tensor("out", [NCH, P, D], bf, kind="ExternalOutput").ap()

    # A2A bounce: shard j = (batch j//4, 128-q block j%4),
    # [CD rows, 128 q] contiguous 32KB.
    obounce = nc.dram_tensor("obounce", [NCH, B, NCH, CD, P], bf).ap()
    # A2A outputs: [src core (= D-row block), CD, q] per chunk
    a2ag = [nc.dram_tensor(f"a2ag{c}", [NCORES, CD, P], bf).ap()
            for c in range(NCH)]
    warm_in = nc.dram_tensor("warm_in", [P], bf).ap()
    warm_out = nc.dram_tensor("warm_out", [NCORES, P], bf,
                              addr_space="Shared").ap()

    RG = [[0, 1, 2, 3, 4, 5, 6, 7]]

    with tile.TileContext(nc, num_cores=NCORES) as tc, ExitStack() as ctx:
        const = ctx.enter_context(tc.tile_pool(name="const", bufs=1))
        work = ctx.enter_context(tc.tile_pool(name="work", bufs=3))
        expp = ctx.enter_context(tc.tile_pool(name="expp", bufs=18))
        otkp = ctx.enter_context(tc.tile_pool(name="otkp", bufs=4))
        osbp = ctx.enter_context(tc.tile_pool(name="osbp", bufs=8))
        psum = ctx.enter_context(tc.tile_pool(name="psum", bufs=2,
                                              space="PSUM"))

        # ---- persistent SBUF ----
        xT_sb = const.tile([P, KD, B, T], bf)        # 64 KB/p
        wq_sb = const.tile([P, KD, CD], bf)
        wk_sb = const.tile([P, KD, CD], bf)
        wv_sb = const.tile([P, KD, CD], bf)
        wp_sb = const.tile([P, KD, D], bf)           # full Wp: 16 KB/p
        bq_sb = const.tile([P, 1], f32)
        mask_sb = const.tile([P, P], bf)             # 0/1 lower triangle
        ident_sb = const.tile([P, P], bf)
        qT_sb = const.tile([P, B, T], bf)            # 2 heads stacked
        kT_sb = const.tile([P, B, T], bf)
        v_sb = const.tile([P, KT, B, HPC, DH + 1], bf)

        # warmup collective FIRST in program order so it triggers as soon
        # as the engines boot: it absorbs the CC-stream entry barrier +
        # first-collective staging cost while the input DMAs stream, so
        # the stream is free when chunk 0's A2A arrives.
        nc.gpsimd.collective_compute(
            "AllGather", bass.mybir.AluOpType.bypass,
            replica_groups=RG, ins=[warm_in], outs=[warm_out])
        # prime the ACT exp table-set during the DMA head (the implicit
        # ACT_TABLE_LOAD costs ~1.3us; exp of uninitialized SBUF is
        # harmless and the result is never read)
        actscr = const.tile([P, 1], f32)
        nc.scalar.activation(actscr[:], actscr[:], AF.Exp)
        nc.vector.memset(v_sb[:, :, :, :, DH:DH + 1], 1.0)
        # DMA order = queue order: interleave wq/x in small slices so the
        # first projection matmuls can start as soon as ~1MB has landed
        # (HBM is chip-contended at boot: all 8 cores stream inputs at
        # once); wp (only needed by the tail projection) goes last.
        xT_r = xT.rearrange("(k p) b t -> p k b t", p=P)
        wq_r = wq.rearrange("(k p) c -> p k c", p=P)
        nc.sync.dma_start(wq_sb[:, 0:1], wq_r[:, 0:1])
        nc.sync.dma_start(xT_sb[:, 0:1, 0, 0:512], xT_r[:, 0:1, 0, 0:512])
        nc.sync.dma_start(wq_sb[:, 1:2], wq_r[:, 1:2])
        nc.sync.dma_start(xT_sb[:, 1:2, 0, 0:512], xT_r[:, 1:2, 0, 0:512])
        nc.sync.dma_start(wq_sb[:, 2:4], wq_r[:, 2:4])
        nc.sync.dma_start(xT_sb[:, 2:4, 0, 0:512], xT_r[:, 2:4, 0, 0:512])
        nc.sync.dma_start(wq_sb[:, 4:8], wq_r[:, 4:8])
        nc.sync.dma_start(xT_sb[:, 4:8, 0, 0:512], xT_r[:, 4:8, 0, 0:512])
        nc.sync.dma_start(bq_sb[:], bqp)
        nc.sync.dma_start(wk_sb[:], wk.rearrange("(k p) c -> p k c", p=P))
        nc.sync.dma_start(wv_sb[:], wv.rearrange("(k p) c -> p k c", p=P))
        nc.sync.dma_start(xT_sb[:, :, 1, 0:512], xT_r[:, :, 1, 0:512])
        nc.sync.dma_start(mask_sb[:], maskf)
        nc.sync.dma_start(ident_sb[:], ident)
        # chunk 1 alone (its qkv drips early, during chunk 0); chunks 2-3
        # merged per batch: 2KB source lines, fewer descriptors
        for b in range(B):
            nc.sync.dma_start(xT_sb[:, :, b, 512:1024],
                              xT_r[:, :, b, 512:1024])
        for b in range(B):
            nc.sync.dma_start(xT_sb[:, :, b, 1024:2048],
                              xT_r[:, :, b, 1024:2048])
        nc.sync.dma_start(wp_sb[:], wp.rearrange("(k p) c -> p k c", p=P))

        def qkv_units(t4):
            """Projection work for T-chunk t4 as a list of closures, so
            it can be drip-fed into the attention k-loop (fills the PE
            while ScalarE paces the exp pipeline)."""
            units = []

            psqk_box = {}

            def q_unit(b):
                psqk = psum.tile([P, 1024], f32, tag="big", bufs=3,
                                 name=f"psqk_{t4}_{b}")
                psqk_box[b] = psqk
                for k in range(KD):
                    nc.tensor.matmul(
                        psqk[:, 0:512], wq_sb[:, k, :],
                        xT_sb[:, k, b, t4 * 512:(t4 + 1) * 512],
                        start=(k == 0), stop=(k == KD - 1))
                nc.vector.tensor_scalar_add(
                    qT_sb[:, b, t4 * 512:(t4 + 1) * 512], psqk[:, 0:512],
                    bq_sb[:, 0:1])

            def k_unit(b):
                psqk = psqk_box[b]
                for k in range(KD):
                    nc.tensor.matmul(
                        psqk[:, 512:1024], wk_sb[:, k, :],
                        xT_sb[:, k, b, t4 * 512:(t4 + 1) * 512],
                        start=(k == 0), stop=(k == KD - 1))
                nc.vector.tensor_copy(
                    kT_sb[:, b, t4 * 512:(t4 + 1) * 512], psqk[:, 512:1024])

            def v_unit(b, tt):
                psv = psum.tile([P, 256], f32, tag="big", bufs=3,
                                name=f"psv_{tt}_{b}")
                for k in range(KD):
                    nc.tensor.matmul(
                        psv[:, :CD], xT_sb[:, k, b, tt * P:(tt + 1) * P],
                        wv_sb[:, k, :], start=(k == 0),
                        stop=(k == KD - 1))
                nc.vector.tensor_copy(
                    out=v_sb[:, tt, b, :, 0:DH],
                    in_=psv[:, :CD].rearrange("p (h d) -> p h d", h=HPC))

            for b in range(B):
                units.append((t4, b, lambda b=b: q_unit(b)))
                units.append((t4, b, lambda b=b: k_unit(b)))
                for tt in range(4 * t4, 4 * t4 + 4):
                    units.append((t4, b, lambda b=b, tt=tt: v_unit(b, tt)))
            return units

        def attention_batch(c, b, filler=None, early=None):
            """Causal attention for q-chunk c, batch b (2 heads packed).

            scores for both heads go into one [128,1024] PSUM tile
            (row-packed K=64 matmuls -> halves), one fused exp per key
            tile, AV interleaved with lag 1.  AV accumulates all four
            q-subtiles of each head in one PSUM bank (4 interleaved
            accumulation groups as column ranges).

            `early` (the previous batch's finish + A2A post) is emitted
            after the first two score tiles so its PE transposes overlap
            the DVE normalize instead of stalling the next batch's
            scores in the Tensor queue.  pso is allocated lazily AFTER
            `early` so the tag-"o" PSUM rotation reuses only buffers
            whose readers are already emitted."""
            nkt = 4 * (c + 1)
            exp_tiles = {}
            pso = {}

            def ensure_pso():
                for hh in range(2):
                    pso[hh] = psum.tile([P, 4, DH + 1], f32, tag="o",
                                        name=f"pso_{c}_{b}_{hh}")

            def do_scores(k):
                j = k - 4 * c
                ql = max(j, 0) * P   # causal: only queries >= key tile
                ps_s = psum.tile([P, 1024], f32, tag="big", bufs=3,
                                 name=f"ps_{c}_{b}_{k}")
                for hh in range(2):
                    lo, hi = hh * DH, (hh + 1) * DH
                    nc.tensor.matmul(
                        ps_s[:, hh * 512 + ql:(hh + 1) * 512],
                        kT_sb[lo:hi, b, k * P:(k + 1) * P],
                        qT_sb[lo:hi, b, c * 512 + ql:(c + 1) * 512],
                        start=True, stop=True)
                e = expp.tile([P, 1024], bf, tag="expT",
                              name=f"expT_{c}_{b}_{k}")
                # diagonal tiles: below-diagonal column blocks are never
                # read by AV, but each ACT op has a ~352cyc fixed pipe
                # cost - only skip when the skipped width exceeds it.
                # (exp of stale PSUM cols lands in never-read SBUF.)
                if j == 3:
                    for hh in range(2):
                        lo = hh * 512 + j * P
                        hi = (hh + 1) * 512
                        nc.scalar.activation(e[:, lo:hi], ps_s[:, lo:hi],
                                             AF.Exp)
                elif j == 2:
                    nc.scalar.activation(e[:, 256:1024], ps_s[:, 256:1024],
                                         AF.Exp)
                else:
                    nc.scalar.activation(e[:], ps_s[:], AF.Exp)
                if j >= 0:
                    blks = e[:].rearrange("p (hh q) -> p hh q", hh=2)[
                        :, :, j * P:(j + 1) * P]
                    nc.vector.tensor_mul(
                        blks, blks,
                        mask_sb[:, None, :].to_broadcast([P, 2, P]))
                exp_tiles[k] = e

            def do_av(k):
                # pso[hh] holds 4 interleaved accumulation groups in one
                # PSUM bank; only the first write of the bank (k==0,s==0)
                # may set start (bank-wide has_written clear).  For diag
                # key tiles (k>0) the mask-dependent s==j block goes last
                # so the other AV matmuls never queue behind the DVE mask.
                j = k - 4 * c
                order = list(range(4))
                if k > 0 and 0 <= j < 4:
                    order = [s for s in order if s != j] + [j]
                for hh in range(2):
                    for s in order:
                        if k <= 4 * c + s:
                            nc.tensor.matmul(
                                pso[hh][:, s, :],
                                exp_tiles[k][:, hh * 512 + s * P:
                                             hh * 512 + (s + 1) * P],
                                v_sb[:, k, b, hh, :],
                                start=(k == 0 and s == 0),
                                stop=(k == 4 * c + s),
                                skip_group_check=True)

            do_scores(0)
            do_scores(1)
            if early is not None:
                early()
            ensure_pso()
            do_av(0)
            for k in range(2, nkt + 1):
                if k < nkt:
                    do_scores(k)
                do_av(k - 1)
                if filler is not None and (c == 0 or k < nkt - 1):
                    # no fills near the end of the loop (except chunk 0,
                    # which feeds no collective): the A2A-critical finish
                    # work must not queue behind drip units
                    filler()
            return pso

        def proj_units(c):
            """Output projection for T-chunk c from the gathered A2A
            slice: this core's 128 query rows x all 1024 output columns.
            Emitted only in the tail, when the A2A is (nearly) done."""
            oT_sb = otkp.tile([P, KD, P], bf, tag="oTk", name=f"oTk_{c}")

            def load():
                g = a2ag[c].rearrange("k r q -> r k q")
                nc.sync.dma_start(oT_sb[:, 0:4], g[:, 0:4])
                nc.sync.dma_start(oT_sb[:, 4:8], g[:, 4:8])

            psp_box = {}

            def mms(n):
                if n == 0:
                    psp_box[0] = psum.tile([P, 2, 512], f32, tag="big",
                                           bufs=3, name=f"psp_{c}")
                psp = psp_box[0]
                for k in range(KD):
                    nc.tensor.matmul(
                        psp[:, n, :], oT_sb[:, k, :],
                        wp_sb[:, k, n * 512:(n + 1) * 512],
                        start=(k == 0), stop=(k == KD - 1))

            def fin():
                psp = psp_box[0]
                outsb = work.tile([P, 2, 512], bf, tag="outsb",
                                  name=f"outsb_{c}")
                for n in range(2):
                    nc.vector.tensor_copy(outsb[:, n, :], psp[:, n, :])
                    nc.sync.dma_start(
                        out[c][:, n * 512:(n + 1) * 512], outsb[:, n, :])

            return [load, lambda: mms(0), lambda: mms(1), fin]

        def finish_batch(c, b, pso):
            """normalize -> PE transpose -> A2A bounce buffer."""
            osb = osbp.tile([P, 4, CD], bf, tag="osb",
                            name=f"osb_{c}_{b}")
            for hh in range(2):
                rec = work.tile([P, 4], f32, tag="rec",
                                name=f"rec_{c}_{b}_{hh}")
                nc.vector.reciprocal(rec[:], pso[hh][:, :, DH:DH + 1])
                nc.vector.tensor_mul(
                    osb[:, :, hh * DH:(hh + 1) * DH],
                    pso[hh][:, :, 0:DH],
                    rec[:, :, None].to_broadcast([P, 4, DH]))
            # local PE transpose: the bounce carries oT so the projection
            # needs no DMA transposes.  All 4 transposes land in one PSUM
            # tile (1KB, fits the tag-"o" slot) -> one contiguous copy.
            obT = work.tile([P, 4, P], bf, tag="obT", name=f"obT_{c}_{b}")
            trp4 = psum.tile([P, 4, P], bf, tag="o", name=f"trp_{c}_{b}")
            for s in range(4):
                nc.tensor.transpose(trp4[:, s, :], osb[:, s, :],
                                    ident_sb[:])
            nc.vector.tensor_copy(obT[:], trp4[:])
            # [row, (qblk q)] -> bounce [qblk, row, q]
            nc.sync.dma_start(
                obounce[c, b].rearrange("s r q -> r s q"), obT[:])

        # pipeline: attention(c) paces ScalarE; one A2A per chunk flies
        # while later chunks compute; qkv(c+1) is drip-fed INTO the
        # attention k-loop so the PE fills ScalarE-paced slack.
        pending = []   # (chunk, batch, closure) drip units

        def filler():
            if pending:
                pending.pop(0)[2]()
            if len(pending) > 8:
                pending.pop(0)[2]()

        def drain_for(c, b):
            while any(t == c and bb == b for t, bb, _ in pending):
                pending.pop(0)[2]()

        def post_a2a(c):
            nc.gpsimd.collective_compute(
                "AllToAll", bass.mybir.AluOpType.bypass,
                replica_groups=RG, ins=[obounce[c]], outs=[a2ag[c]])

        # only batch 0's projections block the first scores
        units0 = qkv_units(0)
        for _, _, u in units0[:6]:
            u()
        pending.extend(units0[6:])
        prevbox = []

        def make_early(snapshot):
            def early():
                for pc, pb, ppso in snapshot:
                    finish_batch(pc, pb, ppso)
                    if pb == B - 1:
                        post_a2a(pc)
            return early

        for c in range(NCH):
            if c + 1 < NCH:
                pending.extend(qkv_units(c + 1))
            if c == 3:
                # A2As 0 and 1 completed ~30us ago: their projections can
                # safely drip into chunk 3's ScalarE-paced slack
                for cc in (0, 1):
                    pending.extend((-1, -1, u) for u in proj_units(cc))
            for b in range(B):
                drain_for(c, b)
                early = make_early(list(prevbox)) if prevbox else None
                prevbox.clear()
                pso = attention_batch(c, b, filler=filler, early=early)
                prevbox.append((c, b, pso))
        # the last batch's finish + A2A are emitted immediately: nothing
        # may delay the final collective
        for pc, pb, ppso in prevbox:
            finish_batch(pc, pb, ppso)
            post_a2a(pc)
        while pending:
            pending.pop(0)[2]()
        # tail: chunk 2's projection runs immediately; chunk 3's waits on
        # the last A2A.  The wait is long (slowest-peer skew), so bridging
        # matmuls can't keep the clock warm through it - run the final
        # projection cold rather than burn PE power on a throwaway bridge.
        for u in proj_units(2):
            u()
        for u in proj_units(3):
            u()

    nc.finalize()
    return nc


def _get_nc():
    if "nc" not in _CACHE:
        _CACHE["nc"] = _build_nc()
    return _CACHE["nc"]


def kernel(x, Wq, bq, Wk, bk, Wv, bv, Wp, bp):
    global LAST_RESULT
    from concourse.bass_utils import run_bass_kernel_spmd

    bf16 = np.float16
    x = np.asarray(x, dtype=np.float32)
    Wq = np.asarray(Wq, dtype=np.float32)
    Wk = np.asarray(Wk, dtype=np.float32)
    Wv = np.asarray(Wv, dtype=np.float32)
    Wp = np.asarray(Wp, dtype=np.float32)
    bq = np.asarray(bq, dtype=np.float32)
    bv = np.asarray(bv, dtype=np.float32)
    bp = np.asarray(bp, dtype=np.float32)

    s = DH ** -0.5
    maskf = np.where(
        np.arange(P)[:, None] <= np.arange(P)[None, :], 1.0, 0.0
    ).astype(bf16)
    ident = np.eye(P, dtype=bf16)
    xTg = np.ascontiguousarray(np.stack([x[0].T, x[1].T], axis=1)
                               ).astype(bf16)
    wp16 = np.ascontiguousarray(Wp).astype(bf16)

    in_maps = []
    for r in range(NCORES):
        cols = slice(r * CD, (r + 1) * CD)
        in_maps.append({
            "xT": xTg,
            "wq": (Wq[:, cols] * s).astype(bf16),
            "wk": np.ascontiguousarray(Wk[:, cols]).astype(bf16),
            "wv": np.ascontiguousarray(Wv[:, cols]).astype(bf16),
            "wp": wp16,
            "bqp": np.ascontiguousarray((bq[cols] * s).reshape(P, 1)),
            "maskf": maskf,
            "ident": ident,
        })

    nc = _get_nc()
    extra = {}
    if os.environ.get("KERNEL_TRACE_ALL"):
        extra["trace_cores"] = list(range(NCORES))
    res = run_bass_kernel_spmd(
        nc, in_maps, core_ids=list(range(NCORES)),
        trace=bool(int(os.environ.get("KERNEL_TRACE", "0"))), **extra)
    LAST_RESULT = res

    out = np.empty((B, T, D), dtype=np.float32)
    for r in range(NCORES):
        o = res.results[r]["out"]          # [NCH, 128, 1024] bf16
        br, blk = r // NCH, r % NCH
        for c in range(NCH):
            out[br, c * CHUNK + blk * P:c * CHUNK + (blk + 1) * P, :] = \
                np.asarray(o[c], dtype=np.float32)
    # bias terms that are constant w.r.t. the data path:
    #   v-bias passes through softmax rows (sum=1) -> + bv@Wp; plus bp.
    #   (bk shifts every logit in a row equally -> cancels in softmax.)
    out += (bv @ Wp + bp)[None, None, :]
    return out

